# revision 11
# baseline (speedup 1.0000x reference)
"""Masked-MLP (CorticalColumnMLP) Trainium2 kernel.

Math: out = gelu(x @ (w1*mask1).T, exact) @ (w2*mask2).T

Key structural fact: mask1 zeroes whole rows of w1 and mask2 zeroes whole
columns of w2 (2-of-4 structured sparsity). gelu(0) == 0, so only hidden
units j with mask1-row j kept AND mask2-col j kept contribute to the
output. Sharding therefore selects exactly those hidden units: the device
runs a dense MLP over the ~2065 surviving hidden units (padded to a
multiple of 128 with zeros, which is exact).

Distribution: pure data-parallel over tokens. Each of the 8 cores gets
1/8 of the 8192 tokens and the full (gathered) weight set; outputs are
disjoint token slices, concatenated on the host. No collectives.

Device layout (per core): everything is laid out so every DMA is
contiguous per partition and no transposes are needed on device:
  xt   [128, KD, Tc]   xt[p,k,t] = x[t, k*128+p]         (lhs, K=D on partitions)
  w1d  [JT, 128, KD, 128]  w1d[j,p,k,c] = W1g[j*128+c, k*128+p]
  w2d  [NT, 128, JT, 128]  w2d[n,p,j,c] = W2g.T[j*128+p, n*128+c]
  outt [NT, 128, Tc]   outt[n,p,t] = out[t, n*128+p]
Layer 1 computes hT[j,t] (hidden-major) so layer 2 can contract over
hidden without any transpose.
"""

import os

import numpy as np
import ml_dtypes

import concourse.bass as bass
import concourse.mybir as mybir
import concourse.tile as tile
from concourse import bacc
from concourse.bass import ts
from concourse.bass_utils import run_bass_kernel_spmd

P = 128
TS = 512  # matmul moving free dim / PSUM bank width (fp32)
N_CORES = 8

# "bf16" | "f16" | "f32r" | "f32" — device matmul dtype
MM_DTYPE = os.environ.get("BASS_MLP_DTYPE", "f32r")

_DT = {
    "bf16": mybir.dt.bfloat16,
    "f16": mybir.dt.float16,
    "f32r": mybir.dt.float32r,
    "f32": mybir.dt.float32,
}
_NPDT = {
    "bf16": ml_dtypes.bfloat16,
    "f16": np.float16,
    "f32r": np.float32,
    "f32": np.float32,
}

# result of the last run_bass_kernel_spmd call (for test harness inspection)
LAST_RESULT = None

_NC_CACHE = {}


def _build_nc(D, Hg, Tc, mode, act="Gelu"):
    """Build + compile the per-core Bass program (dense MLP, hidden=Hg)."""
    act_fn = getattr(mybir.ActivationFunctionType, act)
    dt_in = _DT[mode]
    f32 = mybir.dt.float32
    KD = D // P
    JT = Hg // P
    NT = D // P
    nTS = Tc // TS
    assert D % P == 0 and Hg % P == 0 and Tc % TS == 0

    # phase-A wave width (j-tiles processed k-major together). Waves let the
    # first matmuls start as soon as the first x k-chunk lands instead of
    # waiting for all of x; wave*nTS PSUM groups are in flight at once.
    four_byte = mode in ("f32r", "f32")
    JW = 3 if four_byte else 4
    w1_bufs = JW + 1 if four_byte else 2 * JW
    w2_bufs = 2 if four_byte else 4
    o_bufs = 2 if four_byte else 4

    nc = bacc.Bacc("TRN2", target_bir_lowering=False, debug=False,
                   num_devices=N_CORES)
    xt = nc.dram_tensor("xt", [P, KD, Tc], dt_in, kind="ExternalInput")
    w1d = nc.dram_tensor("w1d", [JT, P, KD, P], dt_in, kind="ExternalInput")
    w2d = nc.dram_tensor("w2d", [NT, P, JT, P], dt_in, kind="ExternalInput")
    outt = nc.dram_tensor("outt", [NT, P, Tc], f32, kind="ExternalOutput")

    with tile.TileContext(nc) as tc:
        with (
            tc.tile_pool(name="xp", bufs=1) as xp,
            tc.tile_pool(name="w1p", bufs=w1_bufs) as w1p,
            tc.tile_pool(name="w2p", bufs=w2_bufs) as w2p,
            tc.tile_pool(name="hp", bufs=1) as hp,
            tc.tile_pool(name="op", bufs=o_bufs) as op,
            tc.tile_pool(name="pp", bufs=8, space="PSUM") as pp,
            tc.tile_pool(name="wup", bufs=1) as wup,
        ):
            # PE warm-up: ~120 trivial matmuls on scratch data keep the PE
            # busy from t~1us so the HAM clock-gate opens (1.2->2.4 GHz)
            # before the first real matmul (which waits ~10us on DMAs).
            wu = wup.tile([P, P], dt_in)
            nc.gpsimd.memset(wu, 0.0)
            wups = pp.tile([P, TS], f32, tag="ps", name="warm_ps")
            for _ in range(120):
                nc.tensor.matmul(wups[:, :P], lhsT=wu, rhs=wu,
                                 start=True, stop=True)

            x_tile = xp.tile([P, KD, Tc], dt_in)
            hT = hp.tile([P, JT, Tc], dt_in)

            # Layer 1: hT[j_tile, t] = gelu(sum_k w1.T @ x), in waves of JW
            # j-tiles, k-major so matmuls chase the x k-chunk DMA stream.
            # DMA emission order (single HWDGE ring => FIFO landing):
            # first wave's w1 blocks, then the x chunks in k order.
            waves = []
            j0 = 0
            while j0 < JT:
                jw = 2 if j0 == 0 else JW  # narrow first wave: less weight
                waves.append(list(range(j0, min(j0 + jw, JT))))  # data gates
                j0 += jw                                         # first MM
            first = True
            for js in waves:
                w1ts = {}
                for j in js:
                    w1ts[j] = w1p.tile([P, KD, P], dt_in, tag="w1", name=f"w1t{j}")
                    # scalar (2nd HWDGE ring): w1 lands in parallel with x
                    nc.scalar.dma_start(w1ts[j], w1d[j])
                if first:
                    for k in range(KD):
                        nc.sync.dma_start(x_tile[:, k, :], xt[:, k, :])
                    first = False
                pss = {}
                for j in js:
                    for t in range(nTS):
                        pss[j, t] = pp.tile([P, TS], f32, tag="ps",
                                            name=f"psA{j}_{t}")
                for k in range(KD):
                    for j in js:
                        for t in range(nTS):
                            nc.tensor.matmul(
                                pss[j, t], lhsT=w1ts[j][:, k, :],
                                rhs=x_tile[:, k, ts(t, TS)],
                                start=(k == 0), stop=(k == KD - 1),
                            )
                for j in js:
                    for t in range(nTS):
                        nc.scalar.activation(hT[:, j, ts(t, TS)], pss[j, t],
                                             act_fn)

            # Layer 2: outT[n_tile, t] = sum_j w2g.T @ hT
            for n in range(NT):
                w2t = w2p.tile([P, JT, P], dt_in, tag="w2")
                nc.scalar.dma_start(w2t, w2d[n])
                for t in range(nTS):
                    ps = pp.tile([P, TS], f32, tag="ps")
                    for j in range(JT):
                        nc.tensor.matmul(
                            ps, lhsT=w2t[:, j, :], rhs=hT[:, j, ts(t, TS)],
                            start=(j == 0), stop=(j == JT - 1),
                        )
                    ot = op.tile([P, TS], f32, tag="o")
                    nc.vector.tensor_copy(ot, ps)
                    nc.sync.dma_start(outt[n, :, ts(t, TS)], ot)

    nc.compile()
    return nc


def _get_nc(D, Hg, Tc, mode):
    key = (D, Hg, Tc, mode)
    if key not in _NC_CACHE:
        _NC_CACHE[key] = _build_nc(D, Hg, Tc, mode)
    return _NC_CACHE[key]


def _pack_w1(W1, Hg, npdt):
    # W1 [Hk, D] -> padded [Hg, D] -> w1d[j,p,k,c] = W1[j*P+c, k*P+p]
    Hk, D = W1.shape
    W1p = np.zeros((Hg, D), np.float32)
    W1p[:Hk] = W1
    # [JT, P(c), KD, P(p)] <- transpose of [JT,c,KD,p] from reshape
    a = W1p.reshape(Hg // P, P, D // P, P).transpose(0, 3, 2, 1)
    return np.ascontiguousarray(a).astype(npdt)


def _pack_w2(W2T, Hg, npdt):
    # W2T [Hk, D] (= w2[:, kb].T) -> padded [Hg, D]
    # w2d[n,p,j,c] = W2T[j*P+p, n*P+c]
    Hk, D = W2T.shape
    W2p = np.zeros((Hg, D), np.float32)
    W2p[:Hk] = W2T
    a = W2p.reshape(Hg // P, P, D // P, P).transpose(2, 1, 0, 3)
    return np.ascontiguousarray(a).astype(npdt)


def _pack_x(xc, npdt):
    # xc [Tc, D] -> xt[p,k,t] = xc[t, k*P+p]
    Tc, D = xc.shape
    a = xc.T.reshape(D // P, P, Tc).transpose(1, 0, 2)
    return np.ascontiguousarray(a).astype(npdt)


def kernel(x, w1, w2, mask1, mask2, _trace=False):
    mode = MM_DTYPE
    npdt = _NPDT[mode]

    x = np.asarray(x, np.float32)
    w1 = np.asarray(w1, np.float32)
    w2 = np.asarray(w2, np.float32)
    mask1 = np.asarray(mask1, np.float32)
    mask2 = np.asarray(mask2, np.float32)

    B, S, D = x.shape
    T = B * S
    H = w1.shape[0]
    x2 = x.reshape(T, D)

    # Sharding of the hidden dimension: keep only hidden units whose
    # mask1 row and mask2 column are nonzero (the rest contribute exactly
    # zero). Requires whole-row / whole-column masks, which is what this
    # module's sparsity pattern guarantees; otherwise fall back to dense.
    structured = bool((mask1 == mask1[:, :1]).all()) and bool(
        (mask2 == mask2[:1, :]).all()
    )
    if structured:
        k1 = np.flatnonzero(mask1[:, 0])
        k2 = np.flatnonzero(mask2[0, :])
        kb = np.intersect1d(k1, k2)
        if kb.size == 0:
            return np.zeros((B, S, D), np.float32)
        W1 = w1[kb]             # [Hk, D], mask1 rows are all-ones here
        W2T = w2[:, kb].T       # [Hk, D], mask2 cols are all-ones here
    else:
        W1 = w1 * mask1
        W2T = (w2 * mask2).T
    Hk = W1.shape[0]
    Hg = max(P, ((Hk + P - 1) // P) * P)

    w1d = _pack_w1(W1, Hg, npdt)
    w2d = _pack_w2(W2T, Hg, npdt)

    # Token-parallel over cores; 2 sequential rounds if Tc doesn't divide
    # nicely into TS-sized chunks per core (not the case for T=8192).
    assert T % N_CORES == 0
    Tc = T // N_CORES
    rounds = 1
    while (Tc // rounds) % TS != 0 or (Tc // rounds) == 0:
        rounds *= 2
        assert rounds <= 16
    Tc //= rounds

    nc = _get_nc(D, Hg, Tc, mode)

    out = np.empty((T, D), np.float32)
    global LAST_RESULT
    for r in range(rounds):
        in_maps = []
        for c in range(N_CORES):
            t0 = (r * N_CORES + c) * Tc
            in_maps.append({
                "xt": _pack_x(x2[t0:t0 + Tc], npdt),
                "w1d": w1d,
                "w2d": w2d,
            })
        res = run_bass_kernel_spmd(
            nc, in_maps, core_ids=list(range(N_CORES)), trace=_trace,
        )
        LAST_RESULT = res
        for c in range(N_CORES):
            t0 = (r * N_CORES + c) * Tc
            o = res.results[c]["outt"]  # [NT, P, Tc]
            out[t0:t0 + Tc] = o.reshape(D, Tc).T

    return out.reshape(B, S, D)


# revision 12
# speedup vs baseline: 1.0344x; 1.0344x over previous
"""Masked-MLP (CorticalColumnMLP) Trainium2 kernel.

Math: out = gelu(x @ (w1*mask1).T, exact) @ (w2*mask2).T

Key structural fact: mask1 zeroes whole rows of w1 and mask2 zeroes whole
columns of w2 (2-of-4 structured sparsity). gelu(0) == 0, so only hidden
units j with mask1-row j kept AND mask2-col j kept contribute to the
output. Sharding therefore selects exactly those hidden units: the device
runs a dense MLP over the ~2065 surviving hidden units (padded to a
multiple of 128 with zeros, which is exact).

Distribution: pure data-parallel over tokens. Each of the 8 cores gets
1/8 of the 8192 tokens and the full (gathered) weight set; outputs are
disjoint token slices, concatenated on the host. No collectives.

Device layout (per core): everything is laid out so every DMA is
contiguous per partition and no transposes are needed on device:
  xt   [128, KD, Tc]   xt[p,k,t] = x[t, k*128+p]         (lhs, K=D on partitions)
  w1d  [JT, 128, KD, 128]  w1d[j,p,k,c] = W1g[j*128+c, k*128+p]
  w2d  [NT, 128, JT, 128]  w2d[n,p,j,c] = W2g.T[j*128+p, n*128+c]
  outt [NT, 128, Tc]   outt[n,p,t] = out[t, n*128+p]
Layer 1 computes hT[j,t] (hidden-major) so layer 2 can contract over
hidden without any transpose.
"""

import os

import numpy as np
import ml_dtypes

import concourse.bass as bass
import concourse.mybir as mybir
import concourse.tile as tile
from concourse import bacc
from concourse.bass import ts
from concourse.bass_utils import run_bass_kernel_spmd

P = 128
TS = 512  # matmul moving free dim / PSUM bank width (fp32)
N_CORES = 8

# "bf16" | "f16" | "f32r" | "f32" — device matmul dtype
MM_DTYPE = os.environ.get("BASS_MLP_DTYPE", "f32r")

_DT = {
    "bf16": mybir.dt.bfloat16,
    "f16": mybir.dt.float16,
    "f32r": mybir.dt.float32r,
    "f32": mybir.dt.float32,
}
_NPDT = {
    "bf16": ml_dtypes.bfloat16,
    "f16": np.float16,
    "f32r": np.float32,
    "f32": np.float32,
}

# result of the last run_bass_kernel_spmd call (for test harness inspection)
LAST_RESULT = None

_NC_CACHE = {}


def _build_nc(D, Hg, Tc, mode, act="Gelu"):
    """Build + compile the per-core Bass program (dense MLP, hidden=Hg)."""
    act_fn = getattr(mybir.ActivationFunctionType, act)
    dt_in = _DT[mode]
    f32 = mybir.dt.float32
    KD = D // P
    JT = Hg // P
    NT = D // P
    nTS = Tc // TS
    assert D % P == 0 and Hg % P == 0 and Tc % TS == 0

    # phase-A wave width (j-tiles processed k-major together). Waves let the
    # first matmuls start as soon as the first x k-chunk lands instead of
    # waiting for all of x; wave*nTS PSUM groups are in flight at once.
    four_byte = mode in ("f32r", "f32")
    JW = 3 if four_byte else 4
    w1_bufs = JW + 1 if four_byte else 2 * JW
    w2_bufs = 2 if four_byte else 4
    o_bufs = 2 if four_byte else 4

    nc = bacc.Bacc("TRN2", target_bir_lowering=False, debug=False,
                   num_devices=N_CORES)
    xt = nc.dram_tensor("xt", [P, KD, Tc], dt_in, kind="ExternalInput")
    w1d = nc.dram_tensor("w1d", [JT, P, KD, P], dt_in, kind="ExternalInput")
    w2d = nc.dram_tensor("w2d", [NT, P, JT, P], dt_in, kind="ExternalInput")
    outt = nc.dram_tensor("outt", [NT, P, Tc], f32, kind="ExternalOutput")

    with tile.TileContext(nc) as tc:
        with (
            tc.tile_pool(name="xp", bufs=1) as xp,
            tc.tile_pool(name="w1p", bufs=w1_bufs) as w1p,
            tc.tile_pool(name="w2p", bufs=w2_bufs) as w2p,
            tc.tile_pool(name="hp", bufs=1) as hp,
            tc.tile_pool(name="op", bufs=o_bufs) as op,
            tc.tile_pool(name="pp", bufs=8, space="PSUM") as pp,
            tc.tile_pool(name="wup", bufs=1) as wup,
        ):
            # PE warm-up: ~120 trivial matmuls on scratch data keep the PE
            # busy from t~1us so the HAM clock-gate opens (1.2->2.4 GHz)
            # before the first real matmul (which waits ~10us on DMAs).
            wu = wup.tile([P, P], dt_in)
            nc.gpsimd.memset(wu, 0.0)
            wups = pp.tile([P, TS], f32, tag="ps", name="warm_ps")
            for _ in range(120):
                nc.tensor.matmul(wups[:, :P], lhsT=wu, rhs=wu,
                                 start=True, stop=True)

            x_tile = xp.tile([P, KD, Tc], dt_in)
            hT = hp.tile([P, JT, Tc], dt_in)

            # Layer 1: hT[j_tile, t] = gelu(sum_k w1.T @ x), in waves of JW
            # j-tiles, k-major so matmuls chase the x k-chunk DMA stream.
            # DMA emission order (single HWDGE ring => FIFO landing):
            # first wave's w1 blocks, then the x chunks in k order.
            first = True
            for w0 in range(0, JT, JW):
                js = list(range(w0, min(w0 + JW, JT)))
                w1ts = {}
                for j in js:
                    w1ts[j] = w1p.tile([P, KD, P], dt_in, tag="w1", name=f"w1t{j}")
                    # scalar (2nd HWDGE ring): w1 lands in parallel with x
                    nc.scalar.dma_start(w1ts[j], w1d[j])
                if first:
                    for k in range(KD):
                        nc.sync.dma_start(x_tile[:, k, :], xt[:, k, :])
                    first = False
                pss = {}
                for j in js:
                    for t in range(nTS):
                        pss[j, t] = pp.tile([P, TS], f32, tag="ps",
                                            name=f"psA{j}_{t}")
                for k in range(KD):
                    for j in js:
                        for t in range(nTS):
                            nc.tensor.matmul(
                                pss[j, t], lhsT=w1ts[j][:, k, :],
                                rhs=x_tile[:, k, ts(t, TS)],
                                start=(k == 0), stop=(k == KD - 1),
                            )
                for j in js:
                    for t in range(nTS):
                        nc.scalar.activation(hT[:, j, ts(t, TS)], pss[j, t],
                                             act_fn)

            # Layer 2: outT[n_tile, t] = sum_j w2g.T @ hT
            for n in range(NT):
                w2t = w2p.tile([P, JT, P], dt_in, tag="w2")
                nc.scalar.dma_start(w2t, w2d[n])
                for t in range(nTS):
                    ps = pp.tile([P, TS], f32, tag="ps")
                    for j in range(JT):
                        nc.tensor.matmul(
                            ps, lhsT=w2t[:, j, :], rhs=hT[:, j, ts(t, TS)],
                            start=(j == 0), stop=(j == JT - 1),
                        )
                    ot = op.tile([P, TS], f32, tag="o")
                    nc.vector.tensor_copy(ot, ps)
                    nc.sync.dma_start(outt[n, :, ts(t, TS)], ot)

    nc.compile()
    return nc


def _get_nc(D, Hg, Tc, mode):
    key = (D, Hg, Tc, mode)
    if key not in _NC_CACHE:
        _NC_CACHE[key] = _build_nc(D, Hg, Tc, mode)
    return _NC_CACHE[key]


def _pack_w1(W1, Hg, npdt):
    # W1 [Hk, D] -> padded [Hg, D] -> w1d[j,p,k,c] = W1[j*P+c, k*P+p]
    Hk, D = W1.shape
    W1p = np.zeros((Hg, D), np.float32)
    W1p[:Hk] = W1
    # [JT, P(c), KD, P(p)] <- transpose of [JT,c,KD,p] from reshape
    a = W1p.reshape(Hg // P, P, D // P, P).transpose(0, 3, 2, 1)
    return np.ascontiguousarray(a).astype(npdt)


def _pack_w2(W2T, Hg, npdt):
    # W2T [Hk, D] (= w2[:, kb].T) -> padded [Hg, D]
    # w2d[n,p,j,c] = W2T[j*P+p, n*P+c]
    Hk, D = W2T.shape
    W2p = np.zeros((Hg, D), np.float32)
    W2p[:Hk] = W2T
    a = W2p.reshape(Hg // P, P, D // P, P).transpose(2, 1, 0, 3)
    return np.ascontiguousarray(a).astype(npdt)


def _pack_x(xc, npdt):
    # xc [Tc, D] -> xt[p,k,t] = xc[t, k*P+p]
    Tc, D = xc.shape
    a = xc.T.reshape(D // P, P, Tc).transpose(1, 0, 2)
    return np.ascontiguousarray(a).astype(npdt)


def kernel(x, w1, w2, mask1, mask2, _trace=False):
    mode = MM_DTYPE
    npdt = _NPDT[mode]

    x = np.asarray(x, np.float32)
    w1 = np.asarray(w1, np.float32)
    w2 = np.asarray(w2, np.float32)
    mask1 = np.asarray(mask1, np.float32)
    mask2 = np.asarray(mask2, np.float32)

    B, S, D = x.shape
    T = B * S
    H = w1.shape[0]
    x2 = x.reshape(T, D)

    # Sharding of the hidden dimension: keep only hidden units whose
    # mask1 row and mask2 column are nonzero (the rest contribute exactly
    # zero). Requires whole-row / whole-column masks, which is what this
    # module's sparsity pattern guarantees; otherwise fall back to dense.
    structured = bool((mask1 == mask1[:, :1]).all()) and bool(
        (mask2 == mask2[:1, :]).all()
    )
    if structured:
        k1 = np.flatnonzero(mask1[:, 0])
        k2 = np.flatnonzero(mask2[0, :])
        kb = np.intersect1d(k1, k2)
        if kb.size == 0:
            return np.zeros((B, S, D), np.float32)
        W1 = w1[kb]             # [Hk, D], mask1 rows are all-ones here
        W2T = w2[:, kb].T       # [Hk, D], mask2 cols are all-ones here
    else:
        W1 = w1 * mask1
        W2T = (w2 * mask2).T
    Hk = W1.shape[0]
    Hg = max(P, ((Hk + P - 1) // P) * P)

    w1d = _pack_w1(W1, Hg, npdt)
    w2d = _pack_w2(W2T, Hg, npdt)

    # Token-parallel over cores; 2 sequential rounds if Tc doesn't divide
    # nicely into TS-sized chunks per core (not the case for T=8192).
    assert T % N_CORES == 0
    Tc = T // N_CORES
    rounds = 1
    while (Tc // rounds) % TS != 0 or (Tc // rounds) == 0:
        rounds *= 2
        assert rounds <= 16
    Tc //= rounds

    nc = _get_nc(D, Hg, Tc, mode)

    out = np.empty((T, D), np.float32)
    global LAST_RESULT
    for r in range(rounds):
        in_maps = []
        for c in range(N_CORES):
            t0 = (r * N_CORES + c) * Tc
            in_maps.append({
                "xt": _pack_x(x2[t0:t0 + Tc], npdt),
                "w1d": w1d,
                "w2d": w2d,
            })
        res = run_bass_kernel_spmd(
            nc, in_maps, core_ids=list(range(N_CORES)), trace=_trace,
        )
        LAST_RESULT = res
        for c in range(N_CORES):
            t0 = (r * N_CORES + c) * Tc
            o = res.results[c]["outt"]  # [NT, P, Tc]
            out[t0:t0 + Tc] = o.reshape(D, Tc).T

    return out.reshape(B, S, D)
